# revision 45
# baseline (speedup 1.0000x reference)
"""Trainium2 Bass kernel for multi-head attention (GQA + RoPE + causal).

Problem shapes (hardcoded): x [2, 2048, 2048] f32, w_qkv [3072, 2048],
w_o [2048, 2048], position_ids [2, 2048] int, mask [1,1,2048,2048] causal.

Sharding: 8 cores = 2 batches x 4 KV-head groups. Each core computes, for
one batch b and one kv-group g (4 query heads + 1 kv head):
  - Y^T = (w_shard @ x[b]^T) in "feature-major" layout [f, s] (bf16 matmuls)
  - RoPE on Q^T/K^T (tables precomputed on host from position_ids)
  - causal attention in transposed-score layout S_T[k, q] (no transposes;
    no max subtraction needed at these score magnitudes |s| < ~10)
  - partial o_proj out^T[oc, s] = w_o_slice^T @ A^T  (bf16 partial)
Host sums the 4 partials per batch and transposes back.

Schedule: phase-shifted software pipeline over 512-wide seq slices.
Segment si emits [attn(t=si-1) + o_proj(t=si-2) | proj(si)] so attention
(emitted first = scheduler priority) always has projection and o_proj
matmul chains available as PE filler at its exp-gated stall points.
Softmax denominators avoid the PE entirely (DVE adds of the exp tiles +
gpsimd partition_all_reduce); the pv PSUM banks are released immediately
via unnormalized copies to SBUF, and the reciprocal+normalize runs
lazily a segment later (DVE recip would otherwise stall the in-order
DVE queue behind the gpsimd all_reduce).  exp() is batched across head
pairs to halve ACT instruction overhead.  Attention-facing tensors
(x-slice, K^T, Q^T, V, A^T) are per-slice pool slots so WAR hazards are
slice-granular and reps overlap cleanly in the repeated timing builds.
"""

import math
from contextlib import ExitStack
from dataclasses import dataclass

import numpy as np
import ml_dtypes

import concourse.bass as bass
import concourse.bass_isa as bass_isa
import concourse.tile as tile
from concourse import bacc, mybir
from concourse.masks import make_identity

P = 128
BF16 = mybir.dt.bfloat16
F32 = mybir.dt.float32
BF16_NP = ml_dtypes.bfloat16

# full-size problem constants
B, S_FULL, HID_FULL = 2, 2048, 2048
NH, NKV, HD = 16, 4, 128
NQL_HD = (NH // NKV) * HD  # 512
ROPE_BASE = 10000.0
N_CORES = 8


@dataclass(frozen=True)
class Cfg:
    S: int = S_FULL          # sequence length
    HID: int = HID_FULL      # model dim (contraction for qkv proj)
    NQL: int = NH // NKV     # local query heads per core
    QT: int = 512            # q tile (matmul free dim)

    @property
    def HT(self):            # contraction tiles for qkv proj
        return self.HID // P

    @property
    def NQT(self):           # q tiles per head (also # of s slices)
        return self.S // self.QT

    @property
    def NKT(self):           # k tiles (128 wide)
        return self.S // P

    @property
    def FQK(self):           # 128-blocks of qk features (NQL q heads + 1 k head)
        return self.NQL + 1

    @property
    def OC(self):            # o_proj output features (full hidden)
        return self.HID

    @property
    def TPQ(self):           # k tiles per q tile (causal step)
        return self.QT // P


def emit(ctx: ExitStack, tc: tile.TileContext, cfg: Cfg, io: dict, n_reps: int = 1):
    res = ctx.enter_context(tc.tile_pool(name="res", bufs=1))
    work = ctx.enter_context(tc.tile_pool(name="work", bufs=1))
    ps = ctx.enter_context(tc.tile_pool(name="ps", bufs=1, space="PSUM"))
    for rep in range(n_reps):  # >1 only for timing builds
        # accumulate into outT on reps > 0 so repeats aren't dead-code
        # eliminated by the NEFF compiler (timing builds only)
        emit_once(tc, cfg, io, res, work, ps, accum=(rep > 0))


def emit_once(tc: tile.TileContext, cfg: Cfg, io: dict, res, work, ps, accum=False):
    nc = tc.nc
    S, QT, HT, NQL = cfg.S, cfg.QT, cfg.HT, cfg.NQL
    NS = cfg.NQT  # s slices of size QT
    xT, wqkT, wvT, woT, cosT, sinT, outT, tri = (
        io["xT"], io["wqkT"], io["wvT"], io["woT"], io["cosT"], io["sinT"],
        io["outT"], io["tri"],
    )

    # ---- whole-rep SBUF residents (weights/tables) ----
    wqk_sb = res.tile([P, HT, cfg.FQK * P], BF16, tag="wqk")
    wv_sb = res.tile([P, HT, P], BF16, tag="wv")
    wo_sb = res.tile([P, NQL, cfg.OC], BF16, tag="wo")
    cos_sb = res.tile([P, S], BF16, tag="cos")
    sin_sb = res.tile([P, S], BF16, tag="sin")
    tri_sb = res.tile([P, P], BF16, tag="tri")

    xT_r = xT.rearrange("(ht p) s -> p ht s", p=P)
    wqk_r = wqkT.rearrange("(ht p) f -> p ht f", p=P)
    kf = bass.ts(NQL, P)  # K feature columns of wqk

    # per-slice tiles (pool slots give slice-granular WAR for rep overlap)
    def x_slice(si):
        t_ = work.tile([P, HT, QT], BF16, tag="xsl", bufs=2, name=f"x{si}")
        sl = bass.ts(si, QT)
        half = HT // 2
        nc.sync.dma_start(out=t_[:, 0:half, :], in_=xT_r[:, 0:half, sl])
        nc.sync.dma_start(out=t_[:, half:HT, :], in_=xT_r[:, half:HT, sl])
        return t_

    # ---- DMA issue order = need order (front-load slice 0 blockers) ----
    nc.sync.dma_start(out=wqk_sb[:, :, kf], in_=wqk_r[:, :, kf])
    xsl = [None] * NS
    xsl[0] = x_slice(0)
    nc.sync.dma_start(out=cos_sb[:], in_=cosT[:, :])
    nc.sync.dma_start(out=sin_sb[:], in_=sinT[:, :])
    nc.sync.dma_start(out=wv_sb[:], in_=wvT.rearrange("(ht p) f -> p ht f", p=P))
    nc.sync.dma_start(out=tri_sb[:], in_=tri[:, :])
    # q-head weight columns, one chunk per head (q0 needed first)
    for fi in range(NQL):
        fsl = bass.ts(fi, P)
        nc.sync.dma_start(out=wqk_sb[:, :, fsl], in_=wqk_r[:, :, fsl])
    xsl[1] = x_slice(1)
    nc.sync.dma_start(out=wo_sb[:], in_=woT.rearrange("(fq p) oc -> p fq oc", p=P))
    xsl[2] = x_slice(2)
    xsl[3] = x_slice(3)

    ident_sb = res.tile([P, P], BF16, tag="ident")
    make_identity(nc, ident_sb[:])

    # ---- projection helper (Y^T for one 128-wide feature block) ----
    def proj_block(fslice, si, dst, do_rope, w_sb=None, copy_dve=False):
        w_sb = wqk_sb if w_sb is None else w_sb
        sl = bass.ts(si, QT)
        acc = ps.tile([P, QT], F32, tag="mm", bufs=2, name="acc")
        for hi in range(HT):
            nc.tensor.matmul(
                acc[:], w_sb[:, hi, fslice], xsl[si][:, hi, :],
                start=(hi == 0), stop=(hi == HT - 1),
            )
        y = work.tile([P, QT], BF16, tag="y", bufs=4, name="y")
        if copy_dve:
            nc.vector.tensor_copy(y[:], acc[:])
        else:
            nc.scalar.copy(y[:], acc[:])
        if not do_rope:
            return y
        # rope: out = y*cos + swap_halves(y)*sin'
        # (sin' is pre-negated in its lower half on host).
        # Half-swap via 1-input copies: 2-input DVE ops require equal
        # SBUF base partitions on HW.
        sw = work.tile([P, QT], BF16, tag="sw", bufs=4, name="sw")
        nc.vector.tensor_copy(sw[0:64, :], y[64:128, :])
        nc.vector.tensor_copy(sw[64:128, :], y[0:64, :])
        t1 = work.tile([P, QT], BF16, tag="t1", bufs=4, name="t1")
        nc.vector.tensor_mul(t1[:], sw[:], sin_sb[:, sl])
        t2 = work.tile([P, QT], BF16, tag="t2", bufs=4, name="t2")
        nc.vector.tensor_mul(t2[:], y[:], cos_sb[:, sl])
        nc.vector.tensor_add(dst, t2[:], t1[:])
        return None

    kts = [None] * NS   # per-slice roped K^T [P, QT]
    qts = [None] * NS   # per-slice roped Q^T [P, NQL, QT]
    vs = [None] * NS    # per-slice natural V [P, TPQ, P]
    ats = [None] * NS   # per-t attention out A^T [P, NQL, QT]

    def proj_seg(si):
        with nc.named_scope(f"k_proj_{si}"):
            kts[si] = work.tile([P, QT], BF16, tag="kT", bufs=4, name=f"kT{si}")
            proj_block(kf, si, kts[si][:], True, copy_dve=True)
        with nc.named_scope(f"v_proj_{si}"):
            vs[si] = work.tile([P, cfg.TPQ, P], BF16, tag="v", bufs=4,
                               name=f"v{si}")
            vt = proj_block(slice(0, P), si, None, False, w_sb=wv_sb,
                            copy_dve=True)
            for j in range(cfg.TPQ):
                pst = ps.tile([P, P], BF16, tag="mm", bufs=2, name="pst")
                nc.tensor.transpose(pst[:], vt[:, bass.ts(j, P)], ident_sb[:])
                nc.vector.tensor_copy(vs[si][:, j, :], pst[:])
        with nc.named_scope(f"q_proj_{si}"):
            qts[si] = work.tile([P, NQL, QT], BF16, tag="qT", bufs=4,
                                name=f"qT{si}")
            for fi in range(NQL):
                proj_block(bass.ts(fi, P), si, qts[si][:, fi, :], True)

    # softmax normalizations deferred off the critical path: each entry is
    # (t, h0, au, bc); flushed one head-pair later (recip+mul on DVE would
    # otherwise stall the in-order DVE queue behind the gpsimd all_reduce,
    # and the norm-muls would hold the pv PSUM banks hostage)
    pending_norm = []

    def flush_norms(keep=0):
        while len(pending_norm) > keep:
            t, h0, au, bc = pending_norm.pop(0)
            rc = work.tile([P, 2, QT], F32, tag="rc", bufs=2, name="rc")
            nc.vector.reciprocal(rc[:], bc[:])
            # norm-muls on the (otherwise idle) gpsimd engine: keeps the
            # in-order DVE queue clear for latency-critical attention ops
            nc.gpsimd.tensor_mul(ats[t][:, h0:h0 + 2, :], au[:], rc[:])

    def attn_seg(t, fillers=()):
        nk = (t + 1) * cfg.TPQ  # valid k tiles (causal)
        ats[t] = work.tile([P, NQL, QT], BF16, tag="a", bufs=3, name=f"a{t}")
        fillers = list(fillers)
        with nc.named_scope(f"attn_{t}"):
            for hp in range(NQL // 2):  # head pairs
                h0 = 2 * hp
                pv = [ps.tile([P, QT], F32, tag="pv", bufs=2, name=f"pv{h}")
                      for h in (0, 1)]
                T2 = work.tile([P, 2, QT], BF16, tag="T", bufs=2, name="T2")
                for j in range(nk):
                    # boundary tiles (delta > 0): columns q' < delta are fully
                    # masked, so restrict the whole chain to [delta:QT].
                    d = max(0, j * P - t * QT)
                    s2 = ps.tile([P, 2, QT], F32, tag="s", bufs=2, name="s2")
                    p2 = work.tile([P, 2, QT], BF16, tag="p", bufs=6, name="p2")
                    for h in (0, 1):
                        nc.tensor.matmul(
                            s2[:, h, d:QT],
                            kts[j // cfg.TPQ][:, bass.ts(j % cfg.TPQ, P)],
                            qts[t][:, h0 + h, d:QT],
                            start=True, stop=True,
                        )
                    nc.scalar.activation(
                        p2[:, :, d:QT], s2[:, :, d:QT],
                        mybir.ActivationFunctionType.Exp,
                    )
                    if j * P - t * QT >= 0:
                        # diagonal 128-block keeps q' >= k' + delta, i.e. the
                        # base (delta=0) triangle at offset delta (both heads
                        # in one op via a stride-0 broadcast of the mask)
                        nc.vector.tensor_mul(
                            p2[:, :, d:d + P], p2[:, :, d:d + P],
                            tri_sb[:, 0:P].unsqueeze(1).broadcast_to(
                                (P, 2, P)),
                        )
                    for h in (0, 1):
                        nc.tensor.matmul(
                            pv[h][:, d:QT],
                            vs[j // cfg.TPQ][:, j % cfg.TPQ, :],
                            p2[:, h, d:QT],
                            start=(j == 0), stop=(j == nk - 1),
                        )
                    # denominator accumulation (masked exps are zeroed),
                    # both heads in one DVE op
                    if j == 0:
                        nc.vector.tensor_copy(T2[:], p2[:])
                    else:
                        nc.vector.tensor_add(
                            T2[:, :, d:QT], T2[:, :, d:QT], p2[:, :, d:QT])
                bc = work.tile([P, 2, QT], F32, tag="bc", bufs=3, name="bc")
                nc.gpsimd.partition_all_reduce(
                    bc[:], T2[:], channels=P, reduce_op=bass_isa.ReduceOp.add)
                # release the pv PSUM banks right away: copy unnormalized
                # attention out to SBUF (one head on DVE, one on ACT)
                au = work.tile([P, 2, QT], BF16, tag="au", bufs=3, name="au")
                nc.scalar.copy(au[:, 0, :], pv[0][:])
                nc.scalar.copy(au[:, 1, :], pv[1][:])
                pending_norm.append((t, h0, au, bc))
        for f in fillers:
            f()

    def oproj_chains(t):
        """One closure per oc-chain (4 matmuls + cast; DMA after each 4)."""
        qsl = bass.ts(t, QT)
        o4s = {}

        def chain(og, k):
            oi = og * 4 + k
            if k == 0:
                o4s[og] = work.tile([P, 4, QT], BF16, tag="o4", bufs=3,
                                    name="o4")
            o4 = o4s[og]
            acc = ps.tile([P, QT], F32, tag="mm", bufs=2, name="acc_o")
            for fi in range(NQL):
                nc.tensor.matmul(
                    acc[:], wo_sb[:, fi, bass.ts(oi, P)], ats[t][:, fi, :],
                    start=(fi == 0), stop=(fi == NQL - 1),
                )
            if accum and oi == 0 and t == 0:
                # timing builds: chain on previous rep's output so the
                # NEFF compiler can't dead-code-eliminate earlier reps
                prev = work.tile([P, QT], BF16, tag="prev", bufs=1,
                                 name="prev")
                nc.sync.dma_start(out=prev[:], in_=outT[0:P, 0:QT])
                nc.vector.tensor_add(o4[:, k, :], acc[:], prev[:])
            else:
                nc.vector.tensor_copy(o4[:, k, :], acc[:])
            if k == 3:
                nc.sync.dma_start(
                    out=outT.rearrange("(oi p) s -> p oi s", p=P)[
                        :, og * 4:og * 4 + 4, qsl],
                    in_=o4[:],
                )

        return [lambda og=og, k=k: chain(og, k)
                for og in range(cfg.OC // (4 * P)) for k in range(4)]

    # ---- phase-shifted pipeline ----
    # segment si: [attn(si-1) + interleaved o_proj(si-2) | proj(si)];
    # +2 drain segments.  attn is emitted first (scheduler priority); the
    # always-ready o_proj(si-2) chains are interleaved into the attention
    # j-loop as PE filler for its exp-gated stall points (critically in the
    # last attention segment, which has no projection work left).
    for si in range(NS + 2):
        fillers = []
        if si > 1:
            fillers = oproj_chains(si - 2)
        if 0 < si <= NS:
            attn_seg(si - 1, fillers)
        else:
            for f in fillers:
                f()
        if si < NS:
            proj_seg(si)
        # flush norms for attn(si-1) at segment end: the all_reduces have a
        # whole proj/oproj emission to drain on Pool (no DVE recip wait), and
        # o_proj(si-1) only consumes them in segment si+2
        flush_norms(keep=0)


def build(cfg: Cfg, n_reps: int = 1):
    nc = bacc.Bacc("TRN2", target_bir_lowering=False, debug=False)
    io = {
        "xT": nc.dram_tensor("xT", [cfg.HID, cfg.S], BF16, kind="ExternalInput").ap(),
        "wqkT": nc.dram_tensor("wqkT", [cfg.HID, cfg.FQK * P], BF16, kind="ExternalInput").ap(),
        "wvT": nc.dram_tensor("wvT", [cfg.HID, P], BF16, kind="ExternalInput").ap(),
        "woT": nc.dram_tensor("woT", [cfg.NQL * P, cfg.OC], BF16, kind="ExternalInput").ap(),
        "cosT": nc.dram_tensor("cosT", [P, cfg.S], BF16, kind="ExternalInput").ap(),
        "sinT": nc.dram_tensor("sinT", [P, cfg.S], BF16, kind="ExternalInput").ap(),
        "tri": nc.dram_tensor("tri", [P, P], BF16, kind="ExternalInput").ap(),
        "outT": nc.dram_tensor("outT", [cfg.OC, cfg.S], BF16, kind="ExternalOutput").ap(),
    }
    with tile.TileContext(nc) as tc:
        with ExitStack() as ctx:
            emit(ctx, tc, cfg, io, n_reps=n_reps)
    nc.compile()
    return nc


def rope_tables(position_ids_b: np.ndarray):
    """cos/sin tables in [d, s] layout, both halves stacked; sin lower half
    negated (so rope = y*cos + swap(y)*sin)."""
    half = HD // 2
    inv_freq = 1.0 / (ROPE_BASE ** (np.arange(half, dtype=np.float64) / half))
    freqs = np.asarray(position_ids_b, dtype=np.float64)[None, :] * inv_freq[:, None]
    cos = np.cos(freqs)
    sin = np.sin(freqs)
    cosT = np.concatenate([cos, cos], 0)
    sinT = np.concatenate([-sin, sin], 0)
    return cosT, sinT


def make_in_maps(x, position_ids, w_qkv, w_o):
    """Shard full inputs into per-core input maps (host-side prep)."""
    q_dim = NH * HD
    kv_dim = NKV * HD
    in_maps = []
    tri = make_tri()
    scale = 1.0 / math.sqrt(HD)
    tabs = {}
    for b in range(B):
        cosT, sinT = rope_tables(position_ids[b])
        tabs[b] = (cosT.astype(BF16_NP), sinT.astype(BF16_NP))
    for c in range(N_CORES):
        b, g = divmod(c, NKV)
        # weights for this core's heads: 4 q heads (pre-scaled), 1 k, 1 v head
        wq = w_qkv[g * NQL_HD:(g + 1) * NQL_HD, :] * scale
        wk = w_qkv[q_dim + g * HD:q_dim + (g + 1) * HD, :]
        wv = w_qkv[q_dim + kv_dim + g * HD:q_dim + kv_dim + (g + 1) * HD, :]
        wqkT = np.ascontiguousarray(np.concatenate([wq, wk], 0).T).astype(BF16_NP)
        wvT = np.ascontiguousarray(wv.T).astype(BF16_NP)
        # o_proj: rows of w_o^T for this core's flattened head features
        woT = np.ascontiguousarray(w_o.T[g * NQL_HD:(g + 1) * NQL_HD, :]).astype(BF16_NP)
        in_maps.append({
            "xT": np.ascontiguousarray(x[b].T).astype(BF16_NP),
            "wqkT": wqkT,
            "wvT": wvT,
            "woT": woT,
            "cosT": tabs[b][0],
            "sinT": tabs[b][1],
            "tri": tri,
        })
    return in_maps


def make_tri():
    """Base boundary mask: tri[k, q] = 1 if q >= k (q, k in [0, 128))."""
    k = np.arange(P)
    q = np.arange(P)
    return (q[None, :] >= k[:, None]).astype(BF16_NP)


def _causal_mask_ok(mask):
    m = np.asarray(mask)
    if m.shape != (1, 1, S_FULL, S_FULL):
        return False
    tril = np.tril(np.ones((S_FULL, S_FULL), dtype=bool))
    m0 = m[0, 0]
    return bool((m0[tril] == 0.0).all() and (m0[~tril] <= -1e8).all())


def _reference_numpy(x, position_ids, mask, w_qkv, w_o):
    """Fallback (never expected to trigger): plain numpy reference."""
    half = HD // 2

    def rope(v, pos):
        inv_freq = 1.0 / (ROPE_BASE ** (np.arange(half) / half))
        f = np.asarray(pos, dtype=np.float64)[:, None] * inv_freq[None, :]
        cos, sin = np.cos(f), np.sin(f)
        x1, x2 = v[..., :half], v[..., half:]
        return np.concatenate([x1 * cos - x2 * sin, x2 * cos + x1 * sin], -1)

    out = np.empty((B, S_FULL, HID_FULL), np.float32)
    q_dim, kv_dim = NH * HD, NKV * HD
    xd = x.astype(np.float64)
    for b in range(B):
        qkv = xd[b] @ w_qkv.T.astype(np.float64)
        q = qkv[:, :q_dim].reshape(S_FULL, NH, HD).transpose(1, 0, 2)
        k = qkv[:, q_dim:q_dim + kv_dim].reshape(S_FULL, NKV, HD).transpose(1, 0, 2)
        v = qkv[:, q_dim + kv_dim:].reshape(S_FULL, NKV, HD).transpose(1, 0, 2)
        q = np.stack([rope(qh, position_ids[b]) for qh in q])
        k = np.stack([rope(kh, position_ids[b]) for kh in k])
        rep = NH // NKV
        acc = np.empty((S_FULL, NH, HD))
        for h in range(NH):
            s = q[h] @ k[h // rep].T / math.sqrt(HD) + mask[0, 0]
            s -= s.max(-1, keepdims=True)
            e = np.exp(s)
            p = e / e.sum(-1, keepdims=True)
            acc[:, h, :] = p @ v[h // rep]
        out[b] = (acc.reshape(S_FULL, NH * HD) @ w_o.T.astype(np.float64)).astype(np.float32)
    return out


_NC_CACHE = {}


def _get_nc():
    if "full" not in _NC_CACHE:
        _NC_CACHE["full"] = build(Cfg())
    return _NC_CACHE["full"]


def kernel(x, position_ids, mask, w_qkv, w_o):
    x = np.asarray(x, dtype=np.float32)
    position_ids = np.asarray(position_ids)
    w_qkv = np.asarray(w_qkv, dtype=np.float32)
    w_o = np.asarray(w_o, dtype=np.float32)
    if not _causal_mask_ok(mask):
        return _reference_numpy(x, position_ids, np.asarray(mask, np.float32),
                                w_qkv, w_o)

    from concourse.bass_utils import run_bass_kernel_spmd

    nc = _get_nc()
    in_maps = make_in_maps(x, position_ids, w_qkv, w_o)
    res = run_bass_kernel_spmd(nc, in_maps, list(range(N_CORES)))
    out = np.empty((B, S_FULL, HID_FULL), dtype=np.float32)
    for b in range(B):
        acc = res.results[b * NKV + 0]["outT"].astype(np.float32)
        for g in range(1, NKV):
            acc = acc + res.results[b * NKV + g]["outT"].astype(np.float32)
        out[b] = acc.T
    return out


# revision 49
# speedup vs baseline: 1.0597x; 1.0597x over previous
"""Trainium2 Bass kernel for multi-head attention (GQA + RoPE + causal).

Problem shapes (hardcoded): x [2, 2048, 2048] f32, w_qkv [3072, 2048],
w_o [2048, 2048], position_ids [2, 2048] int, mask [1,1,2048,2048] causal.

Sharding: 8 cores = 2 batches x 4 KV-head groups. Each core computes, for
one batch b and one kv-group g (4 query heads + 1 kv head):
  - Y^T = (w_shard @ x[b]^T) in "feature-major" layout [f, s] (bf16 matmuls)
  - RoPE on Q^T/K^T (tables precomputed on host from position_ids)
  - causal attention in transposed-score layout S_T[k, q] (no transposes;
    no max subtraction needed at these score magnitudes |s| < ~10)
  - partial o_proj out^T[oc, s] = w_o_slice^T @ A^T  (bf16 partial)
Host sums the 4 partials per batch and transposes back.

Schedule: phase-shifted software pipeline over 512-wide seq slices.
Segment si emits [attn(t=si-1) + o_proj(t=si-2) | proj(si)] so attention
(emitted first = scheduler priority) always has projection and o_proj
matmul chains available as PE filler at its exp-gated stall points.
Softmax denominators avoid the PE entirely (DVE adds of the exp tiles +
gpsimd partition_all_reduce); the pv PSUM banks are released immediately
via unnormalized copies to SBUF, and the reciprocal+normalize runs
lazily a segment later (DVE recip would otherwise stall the in-order
DVE queue behind the gpsimd all_reduce).  exp() is batched across head
pairs to halve ACT instruction overhead.  Attention-facing tensors
(x-slice, K^T, Q^T, V, A^T) are per-slice pool slots so WAR hazards are
slice-granular and reps overlap cleanly in the repeated timing builds.
"""

import math
from contextlib import ExitStack
from dataclasses import dataclass

import numpy as np
import ml_dtypes

import concourse.bass as bass
import concourse.bass_isa as bass_isa
import concourse.tile as tile
from concourse import bacc, mybir
from concourse.masks import make_identity

P = 128
BF16 = mybir.dt.bfloat16
F32 = mybir.dt.float32
BF16_NP = ml_dtypes.bfloat16

# full-size problem constants
B, S_FULL, HID_FULL = 2, 2048, 2048
NH, NKV, HD = 16, 4, 128
NQL_HD = (NH // NKV) * HD  # 512
ROPE_BASE = 10000.0
N_CORES = 8


@dataclass(frozen=True)
class Cfg:
    S: int = S_FULL          # sequence length
    HID: int = HID_FULL      # model dim (contraction for qkv proj)
    NQL: int = NH // NKV     # local query heads per core
    QT: int = 512            # q tile (matmul free dim)

    @property
    def HT(self):            # contraction tiles for qkv proj
        return self.HID // P

    @property
    def NQT(self):           # q tiles per head (also # of s slices)
        return self.S // self.QT

    @property
    def NKT(self):           # k tiles (128 wide)
        return self.S // P

    @property
    def FQK(self):           # 128-blocks of qk features (NQL q heads + 1 k head)
        return self.NQL + 1

    @property
    def OC(self):            # o_proj output features (full hidden)
        return self.HID

    @property
    def TPQ(self):           # k tiles per q tile (causal step)
        return self.QT // P


def emit(ctx: ExitStack, tc: tile.TileContext, cfg: Cfg, io: dict, n_reps: int = 1):
    res = ctx.enter_context(tc.tile_pool(name="res", bufs=1))
    work = ctx.enter_context(tc.tile_pool(name="work", bufs=1))
    ps = ctx.enter_context(tc.tile_pool(name="ps", bufs=1, space="PSUM"))
    for rep in range(n_reps):  # >1 only for timing builds
        # accumulate into outT on reps > 0 so repeats aren't dead-code
        # eliminated by the NEFF compiler (timing builds only)
        emit_once(tc, cfg, io, res, work, ps, accum=(rep > 0))


def emit_once(tc: tile.TileContext, cfg: Cfg, io: dict, res, work, ps, accum=False):
    nc = tc.nc
    S, QT, HT, NQL = cfg.S, cfg.QT, cfg.HT, cfg.NQL
    NS = cfg.NQT  # s slices of size QT
    xT, wqkT, wvT, woT, cosT, sinT, outT, tri = (
        io["xT"], io["wqkT"], io["wvT"], io["woT"], io["cosT"], io["sinT"],
        io["outT"], io["tri"],
    )

    # ---- whole-rep SBUF residents (weights/tables) ----
    wqk_sb = res.tile([P, HT, cfg.FQK * P], BF16, tag="wqk")
    wv_sb = res.tile([P, HT, P], BF16, tag="wv")
    wo_sb = res.tile([P, NQL, cfg.OC], BF16, tag="wo")
    cos_sb = res.tile([P, S], BF16, tag="cos")
    sin_sb = res.tile([P, S], BF16, tag="sin")
    tri_sb = res.tile([P, P], BF16, tag="tri")

    xT_r = xT.rearrange("(ht p) s -> p ht s", p=P)
    wqk_r = wqkT.rearrange("(ht p) f -> p ht f", p=P)
    kf = bass.ts(NQL, P)  # K feature columns of wqk

    # per-slice tiles (pool slots give slice-granular WAR for rep overlap)
    def x_slice(si):
        t_ = work.tile([P, HT, QT], BF16, tag="xsl", bufs=2, name=f"x{si}")
        sl = bass.ts(si, QT)
        half = HT // 2
        nc.sync.dma_start(out=t_[:, 0:half, :], in_=xT_r[:, 0:half, sl])
        nc.sync.dma_start(out=t_[:, half:HT, :], in_=xT_r[:, half:HT, sl])
        return t_

    # ---- DMA issue order = need order (front-load slice 0 blockers) ----
    nc.sync.dma_start(out=wqk_sb[:, :, kf], in_=wqk_r[:, :, kf])
    xsl = [None] * NS
    xsl[0] = x_slice(0)
    nc.sync.dma_start(out=cos_sb[:], in_=cosT[:, :])
    nc.sync.dma_start(out=sin_sb[:], in_=sinT[:, :])
    nc.sync.dma_start(out=wv_sb[:], in_=wvT.rearrange("(ht p) f -> p ht f", p=P))
    nc.sync.dma_start(out=tri_sb[:], in_=tri[:, :])
    # q-head weight columns, one chunk per head (q0 needed first)
    for fi in range(NQL):
        fsl = bass.ts(fi, P)
        nc.sync.dma_start(out=wqk_sb[:, :, fsl], in_=wqk_r[:, :, fsl])
    xsl[1] = x_slice(1)
    nc.sync.dma_start(out=wo_sb[:], in_=woT.rearrange("(fq p) oc -> p fq oc", p=P))
    xsl[2] = x_slice(2)
    xsl[3] = x_slice(3)

    ident_sb = res.tile([P, P], BF16, tag="ident")
    make_identity(nc, ident_sb[:])

    # ---- projection helper (Y^T for one 128-wide feature block) ----
    def proj_block(fslice, si, dst, do_rope, w_sb=None, copy_dve=False):
        w_sb = wqk_sb if w_sb is None else w_sb
        sl = bass.ts(si, QT)
        acc = ps.tile([P, QT], F32, tag="mm", bufs=2, name="acc")
        for hi in range(HT):
            nc.tensor.matmul(
                acc[:], w_sb[:, hi, fslice], xsl[si][:, hi, :],
                start=(hi == 0), stop=(hi == HT - 1),
            )
        y = work.tile([P, QT], BF16, tag="y", bufs=4, name="y")
        if copy_dve:
            nc.vector.tensor_copy(y[:], acc[:])
        else:
            nc.scalar.copy(y[:], acc[:])
        if not do_rope:
            return y
        # rope: out = y*cos + swap_halves(y)*sin'
        # (sin' is pre-negated in its lower half on host).
        # Half-swap via 1-input copies: 2-input DVE ops require equal
        # SBUF base partitions on HW.
        sw = work.tile([P, QT], BF16, tag="sw", bufs=4, name="sw")
        nc.vector.tensor_copy(sw[0:64, :], y[64:128, :])
        nc.vector.tensor_copy(sw[64:128, :], y[0:64, :])
        t1 = work.tile([P, QT], BF16, tag="t1", bufs=4, name="t1")
        nc.vector.tensor_mul(t1[:], sw[:], sin_sb[:, sl])
        t2 = work.tile([P, QT], BF16, tag="t2", bufs=4, name="t2")
        nc.vector.tensor_mul(t2[:], y[:], cos_sb[:, sl])
        nc.vector.tensor_add(dst, t2[:], t1[:])
        return None

    kts = [None] * NS   # per-slice roped K^T [P, QT]
    qts = [None] * NS   # per-slice roped Q^T [P, NQL, QT]
    vs = [None] * NS    # per-slice natural V [P, TPQ, P]
    ats = [None] * NS   # per-t attention out A^T [P, NQL, QT]

    def proj_seg(si):
        with nc.named_scope(f"k_proj_{si}"):
            kts[si] = work.tile([P, QT], BF16, tag="kT", bufs=4, name=f"kT{si}")
            proj_block(kf, si, kts[si][:], True, copy_dve=True)
        with nc.named_scope(f"v_proj_{si}"):
            vs[si] = work.tile([P, cfg.TPQ, P], BF16, tag="v", bufs=4,
                               name=f"v{si}")
            vt = proj_block(slice(0, P), si, None, False, w_sb=wv_sb,
                            copy_dve=True)
            for j in range(cfg.TPQ):
                pst = ps.tile([P, P], BF16, tag="mm", bufs=2, name="pst")
                nc.tensor.transpose(pst[:], vt[:, bass.ts(j, P)], ident_sb[:])
                nc.vector.tensor_copy(vs[si][:, j, :], pst[:])
        with nc.named_scope(f"q_proj_{si}"):
            qts[si] = work.tile([P, NQL, QT], BF16, tag="qT", bufs=4,
                                name=f"qT{si}")
            for fi in range(NQL):
                proj_block(bass.ts(fi, P), si, qts[si][:, fi, :], True)

    # softmax normalizations deferred off the critical path: each entry is
    # (t, h0, au, bc); flushed one head-pair later (recip+mul on DVE would
    # otherwise stall the in-order DVE queue behind the gpsimd all_reduce,
    # and the norm-muls would hold the pv PSUM banks hostage)
    pending_norm = []

    def flush_norms(keep=0):
        while len(pending_norm) > keep:
            t, h0, au, bc = pending_norm.pop(0)
            rc = work.tile([P, 2, QT], F32, tag="rc", bufs=2, name="rc")
            nc.vector.reciprocal(rc[:], bc[:])
            # norm-muls on the (otherwise idle) gpsimd engine: keeps the
            # in-order DVE queue clear for latency-critical attention ops
            nc.gpsimd.tensor_mul(ats[t][:, h0:h0 + 2, :], au[:], rc[:])

    def attn_seg(t, fillers=()):
        nk = (t + 1) * cfg.TPQ  # valid k tiles (causal)
        ats[t] = work.tile([P, NQL, QT], BF16, tag="a", bufs=3, name=f"a{t}")
        fillers = list(fillers)
        with nc.named_scope(f"attn_{t}"):
            for hp in range(NQL // 2):  # head pairs
                h0 = 2 * hp
                pv = [ps.tile([P, QT], F32, tag="pv", bufs=2, name=f"pv{h}")
                      for h in (0, 1)]
                T2 = work.tile([P, 2, QT], BF16, tag="T", bufs=2, name="T2")
                for j in range(nk):
                    # boundary tiles (delta > 0): columns q' < delta are fully
                    # masked, so restrict the whole chain to [delta:QT].
                    d = max(0, j * P - t * QT)
                    s2 = ps.tile([P, 2, QT], F32, tag="s", bufs=2, name="s2")
                    p2 = work.tile([P, 2, QT], BF16, tag="p", bufs=6, name="p2")
                    for h in (0, 1):
                        nc.tensor.matmul(
                            s2[:, h, d:QT],
                            kts[j // cfg.TPQ][:, bass.ts(j % cfg.TPQ, P)],
                            qts[t][:, h0 + h, d:QT],
                            start=True, stop=True,
                        )
                    nc.scalar.activation(
                        p2[:, :, d:QT], s2[:, :, d:QT],
                        mybir.ActivationFunctionType.Exp,
                    )
                    if j * P - t * QT >= 0:
                        # diagonal 128-block keeps q' >= k' + delta, i.e. the
                        # base (delta=0) triangle at offset delta (both heads
                        # in one op via a stride-0 broadcast of the mask)
                        nc.vector.tensor_mul(
                            p2[:, :, d:d + P], p2[:, :, d:d + P],
                            tri_sb[:, 0:P].unsqueeze(1).broadcast_to(
                                (P, 2, P)),
                        )
                    for h in (0, 1):
                        nc.tensor.matmul(
                            pv[h][:, d:QT],
                            vs[j // cfg.TPQ][:, j % cfg.TPQ, :],
                            p2[:, h, d:QT],
                            start=(j == 0), stop=(j == nk - 1),
                        )
                    # denominator accumulation (masked exps are zeroed),
                    # both heads in one DVE op
                    if j == 0:
                        nc.vector.tensor_copy(T2[:], p2[:])
                    else:
                        nc.vector.tensor_add(
                            T2[:, :, d:QT], T2[:, :, d:QT], p2[:, :, d:QT])
                bc = work.tile([P, 2, QT], F32, tag="bc", bufs=3, name="bc")
                nc.gpsimd.partition_all_reduce(
                    bc[:], T2[:], channels=P, reduce_op=bass_isa.ReduceOp.add)
                # release the pv PSUM banks right away: copy unnormalized
                # attention out to SBUF (one head on DVE, one on ACT)
                au = work.tile([P, 2, QT], BF16, tag="au", bufs=3, name="au")
                nc.scalar.copy(au[:, 0, :], pv[0][:])
                nc.scalar.copy(au[:, 1, :], pv[1][:])
                pending_norm.append((t, h0, au, bc))
        for f in fillers:
            f()

    def oproj_chains(t):
        """One closure per oc-chain (4 matmuls + cast; DMA after each 4)."""
        qsl = bass.ts(t, QT)
        o4s = {}

        def chain(og, k):
            oi = og * 4 + k
            if k == 0:
                o4s[og] = work.tile([P, 4, QT], BF16, tag="o4", bufs=3,
                                    name="o4")
            o4 = o4s[og]
            acc = ps.tile([P, QT], F32, tag="mm", bufs=2, name="acc_o")
            for fi in range(NQL):
                nc.tensor.matmul(
                    acc[:], wo_sb[:, fi, bass.ts(oi, P)], ats[t][:, fi, :],
                    start=(fi == 0), stop=(fi == NQL - 1),
                )
            if accum and oi == 0 and t == 0:
                # timing builds: chain on previous rep's output so the
                # NEFF compiler can't dead-code-eliminate earlier reps
                prev = work.tile([P, QT], BF16, tag="prev", bufs=1,
                                 name="prev")
                nc.sync.dma_start(out=prev[:], in_=outT[0:P, 0:QT])
                nc.vector.tensor_add(o4[:, k, :], acc[:], prev[:])
            else:
                nc.vector.tensor_copy(o4[:, k, :], acc[:])
            if k == 3:
                nc.sync.dma_start(
                    out=outT.rearrange("(oi p) s -> p oi s", p=P)[
                        :, og * 4:og * 4 + 4, qsl],
                    in_=o4[:],
                )

        return [lambda og=og, k=k: chain(og, k)
                for og in range(cfg.OC // (4 * P)) for k in range(4)]

    # ---- phase-shifted pipeline ----
    # segment si: [attn(si-1) + interleaved o_proj(si-2) | proj(si)];
    # +2 drain segments.  attn is emitted first (scheduler priority); the
    # always-ready o_proj(si-2) chains are interleaved into the attention
    # j-loop as PE filler for its exp-gated stall points (critically in the
    # last attention segment, which has no projection work left).
    for si in range(NS + 2):
        fillers = []
        if si > 1:
            fillers = oproj_chains(si - 2)
        if 0 < si <= NS:
            attn_seg(si - 1, fillers)
        else:
            for f in fillers:
                f()
        if si < NS:
            proj_seg(si)
        # flush norms for attn(si-1) at segment end: the all_reduces have a
        # whole proj/oproj emission to drain on Pool (no DVE recip wait), and
        # o_proj(si-1) only consumes them in segment si+2
        flush_norms(keep=0)


def build(cfg: Cfg, n_reps: int = 1):
    nc = bacc.Bacc("TRN2", target_bir_lowering=False, debug=False)
    io = {
        "xT": nc.dram_tensor("xT", [cfg.HID, cfg.S], BF16, kind="ExternalInput").ap(),
        "wqkT": nc.dram_tensor("wqkT", [cfg.HID, cfg.FQK * P], BF16, kind="ExternalInput").ap(),
        "wvT": nc.dram_tensor("wvT", [cfg.HID, P], BF16, kind="ExternalInput").ap(),
        "woT": nc.dram_tensor("woT", [cfg.NQL * P, cfg.OC], BF16, kind="ExternalInput").ap(),
        "cosT": nc.dram_tensor("cosT", [P, cfg.S], BF16, kind="ExternalInput").ap(),
        "sinT": nc.dram_tensor("sinT", [P, cfg.S], BF16, kind="ExternalInput").ap(),
        "tri": nc.dram_tensor("tri", [P, P], BF16, kind="ExternalInput").ap(),
        "outT": nc.dram_tensor("outT", [cfg.OC, cfg.S], BF16, kind="ExternalOutput").ap(),
    }
    with tile.TileContext(nc) as tc:
        with ExitStack() as ctx:
            emit(ctx, tc, cfg, io, n_reps=n_reps)
    nc.compile()
    return nc


def rope_tables(position_ids_b: np.ndarray):
    """cos/sin tables in [d, s] layout, both halves stacked; sin lower half
    negated (so rope = y*cos + swap(y)*sin)."""
    half = HD // 2
    inv_freq = 1.0 / (ROPE_BASE ** (np.arange(half, dtype=np.float64) / half))
    freqs = np.asarray(position_ids_b, dtype=np.float64)[None, :] * inv_freq[:, None]
    cos = np.cos(freqs)
    sin = np.sin(freqs)
    cosT = np.concatenate([cos, cos], 0)
    sinT = np.concatenate([-sin, sin], 0)
    return cosT, sinT


def make_in_maps(x, position_ids, w_qkv, w_o):
    """Shard full inputs into per-core input maps (host-side prep)."""
    q_dim = NH * HD
    kv_dim = NKV * HD
    in_maps = []
    tri = make_tri()
    scale = 1.0 / math.sqrt(HD)
    tabs = {}
    for b in range(B):
        cosT, sinT = rope_tables(position_ids[b])
        tabs[b] = (cosT.astype(BF16_NP), sinT.astype(BF16_NP))
    for c in range(N_CORES):
        b, g = divmod(c, NKV)
        # weights for this core's heads: 4 q heads (pre-scaled), 1 k, 1 v head
        wq = w_qkv[g * NQL_HD:(g + 1) * NQL_HD, :] * scale
        wk = w_qkv[q_dim + g * HD:q_dim + (g + 1) * HD, :]
        wv = w_qkv[q_dim + kv_dim + g * HD:q_dim + kv_dim + (g + 1) * HD, :]
        wqkT = np.ascontiguousarray(np.concatenate([wq, wk], 0).T).astype(BF16_NP)
        wvT = np.ascontiguousarray(wv.T).astype(BF16_NP)
        # o_proj: rows of w_o^T for this core's flattened head features
        woT = np.ascontiguousarray(w_o.T[g * NQL_HD:(g + 1) * NQL_HD, :]).astype(BF16_NP)
        in_maps.append({
            "xT": np.ascontiguousarray(x[b].T).astype(BF16_NP),
            "wqkT": wqkT,
            "wvT": wvT,
            "woT": woT,
            "cosT": tabs[b][0],
            "sinT": tabs[b][1],
            "tri": tri,
        })
    return in_maps


def make_tri():
    """Base boundary mask: tri[k, q] = 1 if q >= k (q, k in [0, 128))."""
    k = np.arange(P)
    q = np.arange(P)
    return (q[None, :] >= k[:, None]).astype(BF16_NP)


def _causal_mask_ok(mask):
    m = np.asarray(mask)
    if m.shape != (1, 1, S_FULL, S_FULL):
        return False
    tril = np.tril(np.ones((S_FULL, S_FULL), dtype=bool))
    m0 = m[0, 0]
    return bool((m0[tril] == 0.0).all() and (m0[~tril] <= -1e8).all())


def _reference_numpy(x, position_ids, mask, w_qkv, w_o):
    """Fallback (never expected to trigger): plain numpy reference."""
    half = HD // 2

    def rope(v, pos):
        inv_freq = 1.0 / (ROPE_BASE ** (np.arange(half) / half))
        f = np.asarray(pos, dtype=np.float64)[:, None] * inv_freq[None, :]
        cos, sin = np.cos(f), np.sin(f)
        x1, x2 = v[..., :half], v[..., half:]
        return np.concatenate([x1 * cos - x2 * sin, x2 * cos + x1 * sin], -1)

    out = np.empty((B, S_FULL, HID_FULL), np.float32)
    q_dim, kv_dim = NH * HD, NKV * HD
    xd = x.astype(np.float64)
    for b in range(B):
        qkv = xd[b] @ w_qkv.T.astype(np.float64)
        q = qkv[:, :q_dim].reshape(S_FULL, NH, HD).transpose(1, 0, 2)
        k = qkv[:, q_dim:q_dim + kv_dim].reshape(S_FULL, NKV, HD).transpose(1, 0, 2)
        v = qkv[:, q_dim + kv_dim:].reshape(S_FULL, NKV, HD).transpose(1, 0, 2)
        q = np.stack([rope(qh, position_ids[b]) for qh in q])
        k = np.stack([rope(kh, position_ids[b]) for kh in k])
        rep = NH // NKV
        acc = np.empty((S_FULL, NH, HD))
        for h in range(NH):
            s = q[h] @ k[h // rep].T / math.sqrt(HD) + mask[0, 0]
            s -= s.max(-1, keepdims=True)
            e = np.exp(s)
            p = e / e.sum(-1, keepdims=True)
            acc[:, h, :] = p @ v[h // rep]
        out[b] = (acc.reshape(S_FULL, NH * HD) @ w_o.T.astype(np.float64)).astype(np.float32)
    return out


_NC_CACHE = {}


def _get_nc():
    if "full" not in _NC_CACHE:
        _NC_CACHE["full"] = build(Cfg())
    return _NC_CACHE["full"]


def kernel(x, position_ids, mask, w_qkv, w_o):
    x = np.asarray(x, dtype=np.float32)
    position_ids = np.asarray(position_ids)
    w_qkv = np.asarray(w_qkv, dtype=np.float32)
    w_o = np.asarray(w_o, dtype=np.float32)
    if not _causal_mask_ok(mask):
        return _reference_numpy(x, position_ids, np.asarray(mask, np.float32),
                                w_qkv, w_o)

    from concourse.bass_utils import run_bass_kernel_spmd

    nc = _get_nc()
    in_maps = make_in_maps(x, position_ids, w_qkv, w_o)
    res = run_bass_kernel_spmd(nc, in_maps, list(range(N_CORES)))
    out = np.empty((B, S_FULL, HID_FULL), dtype=np.float32)
    for b in range(B):
        acc = res.results[b * NKV + 0]["outT"].astype(np.float32)
        for g in range(1, NKV):
            acc = acc + res.results[b * NKV + g]["outT"].astype(np.float32)
        out[b] = acc.T
    return out
